# revision 25
# baseline (speedup 1.0000x reference)
"""Trainium2 Bass kernel for nn_DecoderBlock (dense_transformer).

Sharding (8 NeuronCores): core c handles batch b = c//4 and head-group
r = c%4 (3 of 12 heads).  Attention is tensor-parallel over heads within
each 4-core batch group; per-512-row-chunk ReduceScatters (overlapped
with the remaining attention compute) combine the out-projection
partials so that core (b, r) ends up owning four interleaved 128-row
strips (rows c*512 + r*128 for chunk c).  LayerNorms and the FFN then
run sequence-parallel on the owned 512 rows with full FFN weights.

All matmul operands are bf16 (PSUM accumulation and the LayerNorm /
residual path stay fp32), which enables the fast-weight-load path and
halves weight DMA.  Attention chunks run in reverse order (3,2,1,0) so
the final ReduceScatter is the cheapest one and its latency hides
behind the first FFN half (rows 256:512, depends only on chunks 3,2).
Diagonal attention blocks are column-trimmed: scores/exp/AV only touch
the un-masked query range, and one [128,128] triangular mask handles
the diagonal sub-block.  Host-side folds: 1/sqrt(dk) into wq, bo+bv@wo
into the residual input, b2 into LN1's additive bias (and -b2@w1 into
b1).  rsqrt for LayerNorm is exp(-0.5*ln(var+eps)) so the Activation
engine stays in the single function table holding Exp/Ln/Copy/Identity
(no 1.3us table reloads).
"""

import numpy as np
import ml_dtypes

import concourse.bass as bass
import concourse.tile as tile
import concourse.mybir as mybir
from concourse import bacc
from concourse.bass_utils import run_bass_kernel_spmd

# Model dims (hardcoded per the problem spec).
B = 2
S = 2048
D = 768
H = 12
DK = 64
DFF = 3072
EPS = 1e-5

NCORES = 8
RANKS = 4                  # cores per batch group
HPC = H // RANKS           # heads per core = 3
HD = HPC * DK              # head features per core = 192
ROWS = S // RANKS          # owned rows per core = 512
P = 128
NBLK = S // P              # 16 row blocks per batch
OBLK = ROWS // P           # 4 owned row strips
KO_D = D // P              # 6 feature chunks of d_model
KO_F = DFF // P            # 24 feature chunks of d_ff
QC = S // 512              # 4 query chunks of 512

F32 = mybir.dt.float32
F32R = mybir.dt.float32r
BF16 = mybir.dt.bfloat16
AF = mybir.ActivationFunctionType
ALU = mybir.AluOpType

_CACHE = {}


def _build():
    from contextlib import ExitStack

    nc = bacc.Bacc(None, target_bir_lowering=False)

    # ---- external I/O ----
    # x pre-transposed on host: feature-major [768, 2048], bf16
    xbT = nc.dram_tensor("xbT", [D, S], BF16, kind="ExternalInput")
    # owned rows + (bo + bv@wo) folded in, fp32
    xown = nc.dram_tensor("xown", [ROWS, D], F32, kind="ExternalInput")
    # q/k weights (q scaled by 1/sqrt(dk)) padded into 4 chunks of 128:
    # [q0 q1 | q2 pad | k0 k1 | k2 pad] so each head's q and k slices sit
    # at matching partition offsets.
    wqk = nc.dram_tensor("wqk", [D, 4 * P], BF16, kind="ExternalInput")
    bqk = nc.dram_tensor("bqk", [4 * P], F32, kind="ExternalInput")
    wvp = nc.dram_tensor("wvp", [D, HD], BF16, kind="ExternalInput")
    wo_s = nc.dram_tensor("wo_s", [HD, D], BF16, kind="ExternalInput")
    w1 = nc.dram_tensor("w1", [D, DFF], BF16, kind="ExternalInput")
    b1 = nc.dram_tensor("b1", [DFF], F32, kind="ExternalInput")
    w2 = nc.dram_tensor("w2", [DFF, D], BF16, kind="ExternalInput")
    g1b = nc.dram_tensor("g1b", [P, D], F32, kind="ExternalInput")
    beb2 = nc.dram_tensor("beb2", [P, D], F32, kind="ExternalInput")
    g2b = nc.dram_tensor("g2b", [P, D], F32, kind="ExternalInput")
    be2b = nc.dram_tensor("be2b", [P, D], F32, kind="ExternalInput")
    ident_in = nc.dram_tensor("ident", [P, P], F32, kind="ExternalInput")
    ones3_in = nc.dram_tensor("ones3", [DK + 1, HD], BF16, kind="ExternalInput")
    trimask_in = nc.dram_tensor("trimask", [P, P], BF16, kind="ExternalInput")
    out = nc.dram_tensor("out", [ROWS, D], F32, kind="ExternalOutput")

    with tile.TileContext(nc) as tc, ExitStack() as outer:
        consts = outer.enter_context(tc.tile_pool(name="consts", bufs=1))
        dram = outer.enter_context(tc.tile_pool(name="dram", bufs=1, space="DRAM"))

        # PSUM frames: vpsum closes after the projections so mm2 (the
        # paired-score banks) can take its banks during attention.
        frP = ExitStack()
        mmpsum = frP.enter_context(tc.tile_pool(name="mmpsum", bufs=2, space="PSUM"))
        accpsum = frP.enter_context(
            tc.tile_pool(name="accpsum", bufs=2, space="PSUM", side="right")
        )
        frV = ExitStack()
        vpsum = frV.enter_context(tc.tile_pool(name="vpsum", bufs=2, space="PSUM"))

        # ===== frame 1 (left): xT + projection weights (freed early) =====
        fr1 = ExitStack()
        wqkv = fr1.enter_context(tc.tile_pool(name="wqkv", bufs=1))
        xtpool = fr1.enter_context(tc.tile_pool(name="xtpool", bufs=1))

        # First-needed loads first so the first matmul starts ~4us in.
        wqk_sb = wqkv.tile([P, KO_D, 4 * P], BF16)
        nc.sync.dma_start(wqk_sb[:], wqk.rearrange("(ko p) m -> p ko m", p=P))
        bqk_sb = consts.tile([P, 4], F32)
        nc.sync.dma_start(bqk_sb[:], bqk.rearrange("(mo p) -> p mo", p=P))
        xT = xtpool.tile([P, KO_D, S], BF16)
        for nq in range(QC):
            nc.sync.dma_start(
                xT[:, :, nq * 512:(nq + 1) * 512],
                xbT[:, nq * 512:(nq + 1) * 512].rearrange("(ko p) s -> p ko s", p=P),
            )
        wvp_sb = wqkv.tile([P, KO_D, HD], BF16)
        nc.sync.dma_start(wvp_sb[:], wvp.rearrange("(ko p) m -> p ko m", p=P))

        # ===== frame A (right): attention working set =====
        frA = ExitStack()
        qkvpool = frA.enter_context(tc.tile_pool(name="qkvpool", bufs=1, side="right"))
        expp = frA.enter_context(tc.tile_pool(name="expp", bufs=3, side="right"))
        rdenp = frA.enter_context(tc.tile_pool(name="rdenp", bufs=1, side="right"))
        ystage = frA.enter_context(tc.tile_pool(name="ystage", bufs=2, side="right"))
        yrpool = frA.enter_context(tc.tile_pool(name="yrpool", bufs=1, side="right"))

        tri_sb = qkvpool.tile([P, P], BF16)
        nc.sync.dma_start(tri_sb[:], trimask_in[:])
        ones3_sb = consts.tile([DK + 1, HD], BF16)
        nc.sync.dma_start(ones3_sb[:], ones3_in[:])
        wo_sb = qkvpool.tile([P, 2, D], BF16)
        nc.sync.dma_start(wo_sb[:, 0, :], wo_s[0:P, :])
        nc.sync.dma_start(wo_sb[0:HD - P, 1, :], wo_s[P:HD, :])
        ident = consts.tile([P, P], F32)
        nc.sync.dma_start(ident[:], ident_in[:])
        b1_sb = consts.tile([P, KO_F], F32)
        nc.sync.dma_start(b1_sb[:], b1.rearrange("(mo p) -> p mo", p=P))
        g1_sb = consts.tile([P, D], F32)
        nc.sync.dma_start(g1_sb[:], g1b[:])
        beb2_sb = consts.tile([P, D], F32)
        nc.sync.dma_start(beb2_sb[:], beb2[:])
        g2_sb = consts.tile([P, D], F32)
        nc.sync.dma_start(g2_sb[:], g2b[:])
        be2_sb = consts.tile([P, D], F32)
        nc.sync.dma_start(be2_sb[:], be2b[:])
        eps_sb = consts.tile([P, 1], F32)
        nc.vector.memset(eps_sb[:], EPS)

        # ---- q/k + v projections, interleaved per 512-row chunk ----
        # qk chunk layout: 0=[q0 q1], 1=[q2 pad], 2=[k0 k1], 3=[k2 pad]
        qk_sb = qkvpool.tile([P, 4, S], BF16)
        v_sb = qkvpool.tile([P, NBLK, HPC, DK + 1], BF16)
        nc.gpsimd.memset(v_sb[:, :, :, DK], 1.0)
        for nq in range(QC):
            for mo in range(4):
                ps = mmpsum.tile([P, 512], F32, tag="mm")
                for ko in range(KO_D):
                    nc.tensor.matmul(
                        ps[:],
                        wqk_sb[:, ko, mo * P:(mo + 1) * P],
                        xT[:, ko, nq * 512:(nq + 1) * 512],
                        start=(ko == 0),
                        stop=(ko == KO_D - 1),
                    )
                nc.vector.tensor_scalar_add(
                    qk_sb[:, mo, nq * 512:(nq + 1) * 512], ps[:],
                    bqk_sb[:, mo:mo + 1],
                )
            for blk in range(4 * nq, 4 * nq + 4):
                psv = vpsum.tile([P, HPC, DK], F32, tag="mmv", name="psv")
                for ko in range(KO_D):
                    nc.tensor.matmul(
                        psv[:],
                        xT[:, ko, blk * P:(blk + 1) * P],
                        wvp_sb[:, ko, :],
                        start=(ko == 0),
                        stop=(ko == KO_D - 1),
                    )
                nc.scalar.copy(v_sb[:, blk, :, 0:DK], psv[:])

        fr1.close()  # xT / projection weights no longer needed
        frV.close()  # release psv banks for the paired-score tiles
        mm2psum = frP.enter_context(
            tc.tile_pool(name="mm2psum", bufs=2, space="PSUM")
        )

        # ===== frame 3 (left): FFN weights (DMA overlaps attention) =====
        fr3 = ExitStack()
        wffn = fr3.enter_context(tc.tile_pool(name="wffn", bufs=1))
        ffnbuf = fr3.enter_context(tc.tile_pool(name="ffnbuf", bufs=1))
        stage3 = fr3.enter_context(tc.tile_pool(name="stage3", bufs=2))
        lnsmall = fr3.enter_context(tc.tile_pool(name="lnsmall", bufs=2))

        xown_sb = wffn.tile([P, OBLK, D], F32)
        nc.sync.dma_start(
            xown_sb[:], xown.rearrange("(blk p) d -> p blk d", p=P)
        )
        w1_sb = wffn.tile([P, KO_D, DFF], BF16)
        nc.sync.dma_start(w1_sb[:], w1.rearrange("(ko p) m -> p ko m", p=P))
        w2_sb = wffn.tile([P, KO_F, D], BF16)
        nc.sync.dma_start(w2_sb[:], w2.rearrange("(ko p) m -> p ko m", p=P))

        # ---- attention, chunk-major in FORWARD order so the serial
        #      per-chunk ReduceScatters pipeline under the remaining
        #      attention compute ----
        y_cc = dram.tile([S, D], BF16)
        y_red = [dram.tile([P, D], BF16, name=f"y_red{c}") for c in range(QC)]
        attn_sb = qkvpool.tile([P, 2, S], BF16)
        yr_sb = {}
        for c in range(QC):
            den3 = rdenp.tile([DK + 1, 512], F32, tag="den3")
            nc.vector.memset(den3[:], 1.0)
            unr = {}
            for h in range(HPC):
                q_mo, q_off = h // 2, (h % 2) * DK
                k_mo, k_off = 2 + h // 2, (h % 2) * DK
                po = accpsum.tile([DK + 1, 512], F32, tag="acc")
                qsl = qk_sb[q_off:q_off + DK, q_mo, c * 512:(c + 1) * 512]
                # full (pre-diagonal) key blocks, two per exp instruction
                for pr in range(2 * c):
                    ps2 = mm2psum.tile([P, 2, 512], F32, tag="mm2", name="ps2")
                    for t in range(2):
                        kb = 2 * pr + t
                        nc.tensor.matmul(
                            ps2[:, t, :],
                            qk_sb[k_off:k_off + DK, k_mo, kb * P:(kb + 1) * P],
                            qsl,
                            start=True,
                            stop=True,
                        )
                    ex2 = expp.tile([P, 2, 512], BF16, tag="exp")
                    nc.scalar.activation(ex2[:], ps2[:], AF.Exp)
                    for t in range(2):
                        kb = 2 * pr + t
                        nc.tensor.matmul(
                            po[:],
                            v_sb[:, kb, h, :],
                            ex2[:, t, :],
                            start=(kb == 0),
                            stop=False,
                        )
                # diagonal key blocks: only the un-masked query range
                for j in range(4):
                    kb = 4 * c + j
                    col0 = j * P
                    pss = mmpsum.tile([P, 512], F32, tag="mm", name="pss")
                    nc.tensor.matmul(
                        pss[:, col0:512],
                        qk_sb[k_off:k_off + DK, k_mo, kb * P:(kb + 1) * P],
                        qk_sb[q_off:q_off + DK, q_mo,
                              c * 512 + col0:(c + 1) * 512],
                        start=True,
                        stop=True,
                    )
                    exd = expp.tile([P, 512], BF16, tag="expd")
                    nc.scalar.activation(exd[:, col0:512], pss[:, col0:512], AF.Exp)
                    nc.vector.tensor_mul(
                        exd[:, col0:col0 + P], exd[:, col0:col0 + P], tri_sb[:]
                    )
                    nc.tensor.matmul(
                        po[:, col0:512],
                        v_sb[:, kb, h, :],
                        exd[:, col0:512],
                        start=(kb == 0),
                        stop=(j == 3),
                    )
                # stash the denominator row (partition 64 -> partition h;
                # builtin DVE copy handles the base shift, the custom approx
                # op below does not) and the unnormalized AV rows
                nc.vector.tensor_copy(den3[32 * h:32 * h + 1, :], po[DK:DK + 1, :])
                unr[h] = rdenp.tile([DK, 512], BF16, tag=f"unr{h}", name=f"unr{h}")
                nc.vector.tensor_copy(unr[h][:], po[0:DK, :])

            # normalize all 3 heads at once: one single-instruction approx
            # reciprocal (input must be partition-base 0), then block-diagonal
            # PE broadcast of the three 1/den rows across the dk partitions.
            rden3 = rdenp.tile([DK + 1, 512], F32, tag="rden3")
            nc.vector.reciprocal_approx_fast(rden3[:], den3[:])
            rdenb3 = rdenp.tile([DK + 1, 512], BF16, tag="rdenb3")
            nc.scalar.copy(rdenb3[:], rden3[:])
            pb_a = mmpsum.tile([P, 512], F32, tag="mm", name="pb_a")
            nc.tensor.matmul(pb_a[:], ones3_sb[:, 0:P], rdenb3[:], start=True, stop=True)
            pb_b = mmpsum.tile([DK, 512], F32, tag="mm", name="pb_b")
            nc.tensor.matmul(pb_b[:], ones3_sb[:, P:HD], rdenb3[:], start=True, stop=True)
            for h, pbs in ((0, pb_a[0:DK, :]), (1, pb_a[DK:P, :]), (2, pb_b[:])):
                a_mo, a_off = (h * DK) // P, (h * DK) % P
                nc.vector.tensor_mul(
                    attn_sb[a_off:a_off + DK, a_mo, c * 512:(c + 1) * 512],
                    unr[h][:],
                    pbs,
                )

            # ---- out projection for this chunk -> DRAM, then chunk RS ----
            for blk in range(4 * c, 4 * c + 4):
                yst = ystage.tile([P, D], BF16, tag="yst")
                for no in range(2):
                    psy = mmpsum.tile([P, 384], F32, tag="mm", name="psy")
                    nc.tensor.matmul(
                        psy[:],
                        attn_sb[:, 0, blk * P:(blk + 1) * P],
                        wo_sb[:, 0, no * 384:(no + 1) * 384],
                        start=True,
                        stop=False,
                    )
                    nc.tensor.matmul(
                        psy[:],
                        attn_sb[0:HD - P, 1, blk * P:(blk + 1) * P],
                        wo_sb[0:HD - P, 1, no * 384:(no + 1) * 384],
                        start=False,
                        stop=True,
                    )
                    if no == 0:
                        nc.scalar.copy(yst[:, 0:384], psy[:])
                    else:
                        nc.vector.tensor_copy(yst[:, 384:768], psy[:])
                nc.sync.dma_start(y_cc[blk * P:(blk + 1) * P, :], yst[:])
            nc.gpsimd.collective_compute(
                "ReduceScatter",
                ALU.add,
                replica_groups=[[0, 1, 2, 3], [4, 5, 6, 7]],
                ins=[y_cc[c * 512:(c + 1) * 512, :]],
                outs=[y_red[c][:]],
            )
            yr_sb[c] = yrpool.tile([P, D], BF16, name=f"yr{c}")
            nc.gpsimd.dma_start(yr_sb[c][:], y_red[c][:])

        frP.close()  # release attention PSUM banks for the FFN pools

        # ===== LN1 + FFN on the 4 owned strips =====
        u_sb = ffnbuf.tile([P, OBLK, D], F32)
        uT = ffnbuf.tile([P, KO_D, ROWS], BF16)
        h_sb = ffnbuf.tile([P, KO_F, ROWS], BF16)

        tpstack = ExitStack()
        tpsum = tpstack.enter_context(
            tc.tile_pool(name="tpsum", bufs=2, space="PSUM", side="right")
        )
        fpsum = tpstack.enter_context(tc.tile_pool(name="fpsum", bufs=1, space="PSUM"))
        hpsum = tpstack.enter_context(tc.tile_pool(name="hpsum", bufs=2, space="PSUM"))

        def strip_ln1(blk):
            # residual + LN1 on owned strip blk, then transpose into uT
            z = u_sb[:, blk, :]
            nc.vector.tensor_add(z, yr_sb[blk][:], xown_sb[:, blk, :])
            _layernorm(nc, lnsmall, z, z, eps_sb, g1_sb, beb2_sb)
            for fo in range(KO_D):
                pst = tpsum.tile([P, P], F32, tag="tp")
                nc.tensor.transpose(pst[:], z[:, fo * P:(fo + 1) * P], ident[:])
                nc.scalar.copy(uT[:, fo, blk * P:(blk + 1) * P], pst[:])

        def ffn_half(half):
            # half 0 -> rows 0:256 (strips 0,1 arrive first), half 1 -> 256:512
            r0 = 256 * half
            blks = (0, 1) if half == 0 else (2, 3)
            psf = {
                (blk, fo): fpsum.tile(
                    [P, 512 - 256 * fo], F32,
                    tag=f"f{i}_{fo}", name=f"psf{blk}_{fo}",
                )
                for i, blk in enumerate(blks)
                for fo in range(2)
            }
            for k in range(KO_F):
                psh = hpsum.tile([P, 256], F32, tag="psh", name="psh")
                for ko in range(KO_D):
                    nc.tensor.matmul(
                        psh[:],
                        w1_sb[:, ko, k * P:(k + 1) * P],
                        uT[:, ko, r0:r0 + 256],
                        start=(ko == 0),
                        stop=(ko == KO_D - 1),
                    )
                nc.scalar.activation(
                    h_sb[:, k, r0:r0 + 256], psh[:],
                    AF.Relu, bias=b1_sb[:, k:k + 1],
                )
                for blk in blks:
                    for fo, sl in ((0, slice(0, 512)), (1, slice(512, 768))):
                        nc.tensor.matmul(
                            psf[(blk, fo)][:],
                            h_sb[:, k, blk * P:(blk + 1) * P],
                            w2_sb[:, k, sl],
                            start=(k == 0),
                            stop=(k == KO_F - 1),
                        )
            for blk in blks:
                for fo, sl in ((0, slice(0, 512)), (1, slice(512, 768))):
                    nc.vector.tensor_add(
                        u_sb[:, blk, sl], u_sb[:, blk, sl], psf[(blk, fo)][:]
                    )

        def strip_out(blk):
            ost = stage3.tile([P, D], F32, tag="ost")
            _layernorm(nc, lnsmall, ost[:], u_sb[:, blk, :], eps_sb, g2_sb, be2_sb)
            nc.gpsimd.dma_start(out[blk * P:(blk + 1) * P, :], ost[:])

        strip_ln1(0)
        strip_ln1(1)
        ffn_half(0)
        strip_ln1(2)
        strip_ln1(3)
        strip_out(0)
        strip_out(1)
        ffn_half(1)
        strip_out(2)
        strip_out(3)

        tpstack.close()
        frA.close()
        fr3.close()

    nc.compile()
    return nc


def _layernorm(nc, pool, zout, z, eps_sb, g_sb, b_sb):
    """zout = LayerNorm(z) * g + b over the free dim (768) of z [128, 768].

    rstd = exp(-0.5 * ln(var + eps)) keeps the Activation engine in the
    single function table that also holds Exp/Identity/Copy/Relu.
    The g-mul / b-add run on GpSimd (SBUF-only) to offload DVE.
    """
    sub = 256
    nsub = D // sub
    stats = pool.tile([P, nsub, nc.vector.BN_STATS_DIM], F32, tag="ln_stats")
    mv = pool.tile([P, nc.vector.BN_AGGR_DIM], F32, tag="ln_mv")
    zr = z.rearrange("p (n s) -> p n s", s=sub)
    for sg in range(nsub):
        nc.vector.bn_stats(stats[:, sg, :], zr[:, sg, :])
    nc.vector.bn_aggr(mv[:], stats[:])
    std = pool.tile([P, 1], F32, tag="ln_std")
    nc.scalar.activation(std[:], mv[:, 1:2], AF.Sqrt, bias=eps_sb[:])
    rstd = pool.tile([P, 1], F32, tag="ln_rstd")
    nc.vector.reciprocal(rstd[:], std[:])
    nc.vector.tensor_scalar(
        zout, z,
        scalar1=mv[:, 0:1],
        scalar2=rstd[:],
        op0=ALU.subtract,
        op1=ALU.mult,
    )
    nc.vector.tensor_mul(zout, zout, g_sb[:])
    nc.vector.tensor_add(zout, zout, b_sb[:])


def _host_inputs(x, wq, bq, wk, bk, wv, bv, wo, bo, w1, b1, w2, b2,
                 g1, be1, g2, be2):
    """Build the per-core input maps (bf16 weights, constant folds)."""
    f = np.float32
    bf = ml_dtypes.bfloat16
    ident = np.eye(P, dtype=f)
    # triangular diagonal-block mask: keep key p <= query q (within block)
    trimask = (np.arange(P)[:, None] <= np.arange(P)[None, :]).astype(bf)

    xT = [np.ascontiguousarray(x[b].T).astype(bf) for b in range(B)]

    scale = f(1.0 / np.sqrt(DK))
    # The on-chip u tile stores x1 + b2 (b2 rides on LN1's output bias so
    # LN2's residual input is ready); compensate in FFN1's bias.
    b1f = (b1 - b2 @ w1).astype(f)
    bebb = np.broadcast_to(be1 + b2, (P, D)).astype(f)
    resid_bias = (bo + bv @ wo).astype(f)  # bv folded through out-proj

    shared = {
        "w1": np.ascontiguousarray(w1).astype(bf),
        "b1": b1f,
        "w2": np.ascontiguousarray(w2).astype(bf),
        "g1b": np.broadcast_to(g1, (P, D)).astype(f),
        "beb2": bebb,
        "g2b": np.broadcast_to(g2, (P, D)).astype(f),
        "be2b": np.broadcast_to(be2, (P, D)).astype(f),
        "ident": ident,
        "ones3": (32 * (np.arange(HD)[None, :] // DK) == np.arange(DK + 1)[:, None]).astype(bf),
        "trimask": trimask,
    }

    in_maps = []
    for c in range(NCORES):
        b, r = divmod(c, RANKS)
        hs = slice(r * HD, (r + 1) * HD)
        wvp = wv[:, hs].astype(bf)
        # [q0 q1 | q2 pad | k0 k1 | k2 pad], q pre-scaled by 1/sqrt(dk)
        wqkp = np.zeros((D, 4 * P), f)
        wqkp[:, 0:P] = wq[:, hs][:, 0:P] * scale
        wqkp[:, P:P + DK] = wq[:, hs][:, P:HD] * scale
        wqkp[:, 2 * P:3 * P] = wk[:, hs][:, 0:P]
        wqkp[:, 3 * P:3 * P + DK] = wk[:, hs][:, P:HD]
        bqkp = np.zeros(4 * P, f)
        bqkp[0:P] = bq[hs][0:P] * scale
        bqkp[P:P + DK] = bq[hs][P:HD] * scale
        bqkp[2 * P:3 * P] = bk[hs][0:P]
        bqkp[3 * P:3 * P + DK] = bk[hs][P:HD]
        # owned rows: strip r of each 512-chunk, + (bo + bv@wo)
        xown = np.concatenate(
            [x[b, cc * 512 + r * P: cc * 512 + (r + 1) * P] for cc in range(QC)],
            axis=0,
        ) + resid_bias
        m = {
            "xbT": xT[b],
            "xown": np.ascontiguousarray(xown).astype(f),
            "wqk": wqkp.astype(bf),
            "bqk": bqkp,
            "wvp": wvp,
            "wo_s": np.ascontiguousarray(wo[hs, :]).astype(bf),
        }
        m.update(shared)
        in_maps.append({k: np.ascontiguousarray(v) for k, v in m.items()})
    return in_maps


def _get_nc():
    if "nc" not in _CACHE:
        _CACHE["nc"] = _build()
    return _CACHE["nc"]


def run(inputs, **kw):
    """Run on hardware; returns (output, BassKernelResults)."""
    nc = _get_nc()
    in_maps = _host_inputs(**inputs)
    res = run_bass_kernel_spmd(nc, in_maps, core_ids=list(range(NCORES)), **kw)
    out = np.empty((B, S, D), np.float32)
    for core in range(NCORES):
        b, r = divmod(core, RANKS)
        o = res.results[core]["out"]
        for c in range(QC):
            out[b, c * 512 + r * P: c * 512 + (r + 1) * P, :] = o[c * P:(c + 1) * P]
    return out, res


def kernel(**inputs):
    return run(inputs)[0]


# revision 26
# speedup vs baseline: 1.1584x; 1.1584x over previous
"""Trainium2 Bass kernel for nn_DecoderBlock (dense_transformer).

Sharding (8 NeuronCores): core c handles batch b = c//4 and head-group
r = c%4 (3 of 12 heads).  Attention is tensor-parallel over heads within
each 4-core batch group; per-512-row-chunk ReduceScatters (overlapped
with the remaining attention compute) combine the out-projection
partials so that core (b, r) ends up owning four interleaved 128-row
strips (rows c*512 + r*128 for chunk c).  LayerNorms and the FFN then
run sequence-parallel on the owned 512 rows with full FFN weights.

All matmul operands are bf16 (PSUM accumulation and the LayerNorm /
residual path stay fp32), which enables the fast-weight-load path and
halves weight DMA.  Attention chunks run in reverse order (3,2,1,0) so
the final ReduceScatter is the cheapest one and its latency hides
behind the first FFN half (rows 256:512, depends only on chunks 3,2).
Diagonal attention blocks are column-trimmed: scores/exp/AV only touch
the un-masked query range, and one [128,128] triangular mask handles
the diagonal sub-block.  Host-side folds: 1/sqrt(dk) into wq, bo+bv@wo
into the residual input, b2 into LN1's additive bias (and -b2@w1 into
b1).  rsqrt for LayerNorm is exp(-0.5*ln(var+eps)) so the Activation
engine stays in the single function table holding Exp/Ln/Copy/Identity
(no 1.3us table reloads).
"""

import numpy as np
import ml_dtypes

import concourse.bass as bass
import concourse.tile as tile
import concourse.mybir as mybir
from concourse import bacc
from concourse.bass_utils import run_bass_kernel_spmd

# Model dims (hardcoded per the problem spec).
B = 2
S = 2048
D = 768
H = 12
DK = 64
DFF = 3072
EPS = 1e-5

NCORES = 8
RANKS = 4                  # cores per batch group
HPC = H // RANKS           # heads per core = 3
HD = HPC * DK              # head features per core = 192
ROWS = S // RANKS          # owned rows per core = 512
P = 128
NBLK = S // P              # 16 row blocks per batch
OBLK = ROWS // P           # 4 owned row strips
KO_D = D // P              # 6 feature chunks of d_model
KO_F = DFF // P            # 24 feature chunks of d_ff
QC = S // 512              # 4 query chunks of 512

F32 = mybir.dt.float32
F32R = mybir.dt.float32r
BF16 = mybir.dt.bfloat16
AF = mybir.ActivationFunctionType
ALU = mybir.AluOpType

_CACHE = {}


def _build():
    from contextlib import ExitStack

    nc = bacc.Bacc(None, target_bir_lowering=False)

    # ---- external I/O ----
    # x pre-transposed on host: feature-major [768, 2048], bf16
    xbT = nc.dram_tensor("xbT", [D, S], BF16, kind="ExternalInput")
    # owned rows + (bo + bv@wo) folded in, fp32
    xown = nc.dram_tensor("xown", [ROWS, D], F32, kind="ExternalInput")
    # q/k weights (q scaled by 1/sqrt(dk)) padded into 4 chunks of 128:
    # [q0 q1 | q2 pad | k0 k1 | k2 pad] so each head's q and k slices sit
    # at matching partition offsets.
    wqk = nc.dram_tensor("wqk", [D, 4 * P], BF16, kind="ExternalInput")
    bqk = nc.dram_tensor("bqk", [4 * P], F32, kind="ExternalInput")
    wvp = nc.dram_tensor("wvp", [D, HD], BF16, kind="ExternalInput")
    wo_s = nc.dram_tensor("wo_s", [HD, D], BF16, kind="ExternalInput")
    w1 = nc.dram_tensor("w1", [D, DFF], BF16, kind="ExternalInput")
    b1 = nc.dram_tensor("b1", [DFF], F32, kind="ExternalInput")
    w2 = nc.dram_tensor("w2", [DFF, D], BF16, kind="ExternalInput")
    g1b = nc.dram_tensor("g1b", [P, D], F32, kind="ExternalInput")
    beb2 = nc.dram_tensor("beb2", [P, D], F32, kind="ExternalInput")
    g2b = nc.dram_tensor("g2b", [P, D], F32, kind="ExternalInput")
    be2b = nc.dram_tensor("be2b", [P, D], F32, kind="ExternalInput")
    ident_in = nc.dram_tensor("ident", [P, P], F32, kind="ExternalInput")
    ones3_in = nc.dram_tensor("ones3", [DK + 1, HD], BF16, kind="ExternalInput")
    trimask_in = nc.dram_tensor("trimask", [P, P], BF16, kind="ExternalInput")
    out = nc.dram_tensor("out", [ROWS, D], F32, kind="ExternalOutput")

    with tile.TileContext(nc) as tc, ExitStack() as outer:
        consts = outer.enter_context(tc.tile_pool(name="consts", bufs=1))
        dram = outer.enter_context(tc.tile_pool(name="dram", bufs=1, space="DRAM"))

        # PSUM frames: vpsum closes after the projections so mm2 (the
        # paired-score banks) can take its banks during attention.
        frP = ExitStack()
        mmpsum = frP.enter_context(tc.tile_pool(name="mmpsum", bufs=2, space="PSUM"))
        accpsum = frP.enter_context(
            tc.tile_pool(name="accpsum", bufs=2, space="PSUM", side="right")
        )
        mm2psum = frP.enter_context(
            tc.tile_pool(name="mm2psum", bufs=2, space="PSUM")
        )

        # ===== frame 1 (left): xT + projection weights (freed early) =====
        fr1 = ExitStack()
        wqkv = fr1.enter_context(tc.tile_pool(name="wqkv", bufs=1))
        xtpool = fr1.enter_context(tc.tile_pool(name="xtpool", bufs=1))

        # First-needed loads first so the first matmul starts ~4us in.
        wqk_sb = wqkv.tile([P, KO_D, 4 * P], BF16)
        nc.sync.dma_start(wqk_sb[:], wqk.rearrange("(ko p) m -> p ko m", p=P))
        bqk_sb = consts.tile([P, 4], F32)
        nc.sync.dma_start(bqk_sb[:], bqk.rearrange("(mo p) -> p mo", p=P))
        xT = xtpool.tile([P, KO_D, S], BF16)
        for nq in range(QC):
            nc.sync.dma_start(
                xT[:, :, nq * 512:(nq + 1) * 512],
                xbT[:, nq * 512:(nq + 1) * 512].rearrange("(ko p) s -> p ko s", p=P),
            )
        wvp_sb = wqkv.tile([P, KO_D, HD], BF16)
        nc.sync.dma_start(wvp_sb[:], wvp.rearrange("(ko p) m -> p ko m", p=P))

        # ===== frame A (right): attention working set =====
        frA = ExitStack()
        qkvpool = frA.enter_context(tc.tile_pool(name="qkvpool", bufs=1, side="right"))
        expp = frA.enter_context(tc.tile_pool(name="expp", bufs=3, side="right"))
        rdenp = frA.enter_context(tc.tile_pool(name="rdenp", bufs=1, side="right"))
        ystage = frA.enter_context(tc.tile_pool(name="ystage", bufs=2, side="right"))
        yrpool = frA.enter_context(tc.tile_pool(name="yrpool", bufs=1, side="right"))

        tri_sb = qkvpool.tile([P, P], BF16)
        nc.sync.dma_start(tri_sb[:], trimask_in[:])
        ones3_sb = consts.tile([DK + 1, HD], BF16)
        nc.sync.dma_start(ones3_sb[:], ones3_in[:])
        wo_sb = qkvpool.tile([P, 2, D], BF16)
        nc.sync.dma_start(wo_sb[:, 0, :], wo_s[0:P, :])
        nc.sync.dma_start(wo_sb[0:HD - P, 1, :], wo_s[P:HD, :])
        ident = consts.tile([P, P], F32)
        nc.sync.dma_start(ident[:], ident_in[:])
        b1_sb = consts.tile([P, KO_F], F32)
        nc.sync.dma_start(b1_sb[:], b1.rearrange("(mo p) -> p mo", p=P))
        g1_sb = consts.tile([P, D], F32)
        nc.sync.dma_start(g1_sb[:], g1b[:])
        beb2_sb = consts.tile([P, D], F32)
        nc.sync.dma_start(beb2_sb[:], beb2[:])
        g2_sb = consts.tile([P, D], F32)
        nc.sync.dma_start(g2_sb[:], g2b[:])
        be2_sb = consts.tile([P, D], F32)
        nc.sync.dma_start(be2_sb[:], be2b[:])
        eps_sb = consts.tile([P, 1], F32)
        nc.vector.memset(eps_sb[:], EPS)

        # ---- q/k + v projections, interleaved per 512-row chunk ----
        # qk chunk layout: 0=[q0 q1], 1=[q2 pad], 2=[k0 k1], 3=[k2 pad]
        qk_sb = qkvpool.tile([P, 4, S], BF16)
        v_sb = qkvpool.tile([P, NBLK, HPC, DK + 1], BF16)
        nc.gpsimd.memset(v_sb[:, :, :, DK], 1.0)
        def proj_chunk(nq):
            for mo in range(4):
                ps = mmpsum.tile([P, 512], F32, tag="mm")
                for ko in range(KO_D):
                    nc.tensor.matmul(
                        ps[:],
                        wqk_sb[:, ko, mo * P:(mo + 1) * P],
                        xT[:, ko, nq * 512:(nq + 1) * 512],
                        start=(ko == 0),
                        stop=(ko == KO_D - 1),
                    )
                nc.vector.tensor_scalar_add(
                    qk_sb[:, mo, nq * 512:(nq + 1) * 512], ps[:],
                    bqk_sb[:, mo:mo + 1],
                )
            for blk in range(4 * nq, 4 * nq + 4):
                psv = mmpsum.tile([P, HPC, DK], F32, tag="mm", name="psv")
                for ko in range(KO_D):
                    nc.tensor.matmul(
                        psv[:],
                        xT[:, ko, blk * P:(blk + 1) * P],
                        wvp_sb[:, ko, :],
                        start=(ko == 0),
                        stop=(ko == KO_D - 1),
                    )
                nc.scalar.copy(v_sb[:, blk, :, 0:DK], psv[:])

        # ---- attention, chunk-major in FORWARD order so the serial
        #      per-chunk ReduceScatters pipeline under the remaining
        #      attention compute; chunk c only needs projections <= c, so
        #      proj and attention interleave and RS0/RS1 trigger early ----
        y_cc = dram.tile([S, D], BF16)
        y_red = [dram.tile([P, D], BF16, name=f"y_red{c}") for c in range(QC)]
        attn_sb = qkvpool.tile([P, 2, S], BF16)
        yr_sb = {}

        def attn_chunk(c):
            den3 = rdenp.tile([DK + 1, 512], F32, tag="den3")
            nc.vector.memset(den3[:], 1.0)
            unr = {}
            for h in range(HPC):
                q_mo, q_off = h // 2, (h % 2) * DK
                k_mo, k_off = 2 + h // 2, (h % 2) * DK
                po = accpsum.tile([DK + 1, 512], F32, tag="acc")
                qsl = qk_sb[q_off:q_off + DK, q_mo, c * 512:(c + 1) * 512]
                # full (pre-diagonal) key blocks, two per exp instruction
                for pr in range(2 * c):
                    ps2 = mm2psum.tile([P, 2, 512], F32, tag="mm2", name="ps2")
                    for t in range(2):
                        kb = 2 * pr + t
                        nc.tensor.matmul(
                            ps2[:, t, :],
                            qk_sb[k_off:k_off + DK, k_mo, kb * P:(kb + 1) * P],
                            qsl,
                            start=True,
                            stop=True,
                        )
                    ex2 = expp.tile([P, 2, 512], BF16, tag="exp")
                    nc.scalar.activation(ex2[:], ps2[:], AF.Exp)
                    for t in range(2):
                        kb = 2 * pr + t
                        nc.tensor.matmul(
                            po[:],
                            v_sb[:, kb, h, :],
                            ex2[:, t, :],
                            start=(kb == 0),
                            stop=False,
                        )
                # diagonal key blocks: only the un-masked query range
                for j in range(4):
                    kb = 4 * c + j
                    col0 = j * P
                    pss = mmpsum.tile([P, 512], F32, tag="mm", name="pss")
                    nc.tensor.matmul(
                        pss[:, col0:512],
                        qk_sb[k_off:k_off + DK, k_mo, kb * P:(kb + 1) * P],
                        qk_sb[q_off:q_off + DK, q_mo,
                              c * 512 + col0:(c + 1) * 512],
                        start=True,
                        stop=True,
                    )
                    exd = expp.tile([P, 512], BF16, tag="expd")
                    nc.scalar.activation(exd[:, col0:512], pss[:, col0:512], AF.Exp)
                    nc.vector.tensor_mul(
                        exd[:, col0:col0 + P], exd[:, col0:col0 + P], tri_sb[:]
                    )
                    nc.tensor.matmul(
                        po[:, col0:512],
                        v_sb[:, kb, h, :],
                        exd[:, col0:512],
                        start=(kb == 0),
                        stop=(j == 3),
                    )
                # stash the denominator row (partition 64 -> partition h;
                # builtin DVE copy handles the base shift, the custom approx
                # op below does not) and the unnormalized AV rows
                nc.vector.tensor_copy(den3[32 * h:32 * h + 1, :], po[DK:DK + 1, :])
                unr[h] = rdenp.tile([DK, 512], BF16, tag=f"unr{h}", name=f"unr{h}")
                nc.vector.tensor_copy(unr[h][:], po[0:DK, :])

            # normalize all 3 heads at once: one single-instruction approx
            # reciprocal (input must be partition-base 0), then block-diagonal
            # PE broadcast of the three 1/den rows across the dk partitions.
            rden3 = rdenp.tile([DK + 1, 512], F32, tag="rden3")
            nc.vector.reciprocal_approx_fast(rden3[:], den3[:])
            rdenb3 = rdenp.tile([DK + 1, 512], BF16, tag="rdenb3")
            nc.scalar.copy(rdenb3[:], rden3[:])
            pb_a = mmpsum.tile([P, 512], F32, tag="mm", name="pb_a")
            nc.tensor.matmul(pb_a[:], ones3_sb[:, 0:P], rdenb3[:], start=True, stop=True)
            pb_b = mmpsum.tile([DK, 512], F32, tag="mm", name="pb_b")
            nc.tensor.matmul(pb_b[:], ones3_sb[:, P:HD], rdenb3[:], start=True, stop=True)
            for h, pbs in ((0, pb_a[0:DK, :]), (1, pb_a[DK:P, :]), (2, pb_b[:])):
                a_mo, a_off = (h * DK) // P, (h * DK) % P
                nc.vector.tensor_mul(
                    attn_sb[a_off:a_off + DK, a_mo, c * 512:(c + 1) * 512],
                    unr[h][:],
                    pbs,
                )

            # ---- out projection for this chunk -> DRAM, then chunk RS ----
            for blk in range(4 * c, 4 * c + 4):
                yst = ystage.tile([P, D], BF16, tag="yst")
                for no in range(2):
                    psy = mmpsum.tile([P, 384], F32, tag="mm", name="psy")
                    nc.tensor.matmul(
                        psy[:],
                        attn_sb[:, 0, blk * P:(blk + 1) * P],
                        wo_sb[:, 0, no * 384:(no + 1) * 384],
                        start=True,
                        stop=False,
                    )
                    nc.tensor.matmul(
                        psy[:],
                        attn_sb[0:HD - P, 1, blk * P:(blk + 1) * P],
                        wo_sb[0:HD - P, 1, no * 384:(no + 1) * 384],
                        start=False,
                        stop=True,
                    )
                    if no == 0:
                        nc.scalar.copy(yst[:, 0:384], psy[:])
                    else:
                        nc.vector.tensor_copy(yst[:, 384:768], psy[:])
                nc.sync.dma_start(y_cc[blk * P:(blk + 1) * P, :], yst[:])
            nc.gpsimd.collective_compute(
                "ReduceScatter",
                ALU.add,
                replica_groups=[[0, 1, 2, 3], [4, 5, 6, 7]],
                ins=[y_cc[c * 512:(c + 1) * 512, :]],
                outs=[y_red[c][:]],
            )
            yr_sb[c] = yrpool.tile([P, D], BF16, name=f"yr{c}")
            nc.gpsimd.dma_start(yr_sb[c][:], y_red[c][:])

        proj_chunk(0)
        attn_chunk(0)
        proj_chunk(1)
        attn_chunk(1)
        proj_chunk(2)
        proj_chunk(3)
        fr1.close()  # xT / projection weights no longer needed

        # ===== frame 3 (left): FFN weights (DMA overlaps attention) =====
        fr3 = ExitStack()
        wffn = fr3.enter_context(tc.tile_pool(name="wffn", bufs=1))
        ffnbuf = fr3.enter_context(tc.tile_pool(name="ffnbuf", bufs=1))
        stage3 = fr3.enter_context(tc.tile_pool(name="stage3", bufs=2))
        lnsmall = fr3.enter_context(tc.tile_pool(name="lnsmall", bufs=2))

        xown_sb = wffn.tile([P, OBLK, D], F32)
        nc.sync.dma_start(
            xown_sb[:], xown.rearrange("(blk p) d -> p blk d", p=P)
        )
        w1_sb = wffn.tile([P, KO_D, DFF], BF16)
        nc.sync.dma_start(w1_sb[:], w1.rearrange("(ko p) m -> p ko m", p=P))
        w2_sb = wffn.tile([P, KO_F, D], BF16)
        nc.sync.dma_start(w2_sb[:], w2.rearrange("(ko p) m -> p ko m", p=P))

        attn_chunk(2)
        attn_chunk(3)

        frP.close()  # release attention PSUM banks for the FFN pools

        # ===== LN1 + FFN on the 4 owned strips =====
        u_sb = ffnbuf.tile([P, OBLK, D], F32)
        uT = ffnbuf.tile([P, KO_D, ROWS], BF16)
        h_sb = ffnbuf.tile([P, KO_F, ROWS], BF16)

        tpstack = ExitStack()
        tpsum = tpstack.enter_context(
            tc.tile_pool(name="tpsum", bufs=2, space="PSUM", side="right")
        )
        fpsum = tpstack.enter_context(tc.tile_pool(name="fpsum", bufs=1, space="PSUM"))
        hpsum = tpstack.enter_context(tc.tile_pool(name="hpsum", bufs=2, space="PSUM"))

        def strip_ln1(blk):
            # residual + LN1 on owned strip blk, then transpose into uT
            z = u_sb[:, blk, :]
            nc.vector.tensor_add(z, yr_sb[blk][:], xown_sb[:, blk, :])
            _layernorm(nc, lnsmall, z, z, eps_sb, g1_sb, beb2_sb)
            for fo in range(KO_D):
                pst = tpsum.tile([P, P], F32, tag="tp")
                nc.tensor.transpose(pst[:], z[:, fo * P:(fo + 1) * P], ident[:])
                nc.scalar.copy(uT[:, fo, blk * P:(blk + 1) * P], pst[:])

        def ffn_half(half):
            # half 0 -> rows 0:256 (strips 0,1 arrive first), half 1 -> 256:512
            r0 = 256 * half
            blks = (0, 1) if half == 0 else (2, 3)
            psf = {
                (blk, fo): fpsum.tile(
                    [P, 512 - 256 * fo], F32,
                    tag=f"f{i}_{fo}", name=f"psf{blk}_{fo}",
                )
                for i, blk in enumerate(blks)
                for fo in range(2)
            }
            for k in range(KO_F):
                psh = hpsum.tile([P, 256], F32, tag="psh", name="psh")
                for ko in range(KO_D):
                    nc.tensor.matmul(
                        psh[:],
                        w1_sb[:, ko, k * P:(k + 1) * P],
                        uT[:, ko, r0:r0 + 256],
                        start=(ko == 0),
                        stop=(ko == KO_D - 1),
                    )
                nc.scalar.activation(
                    h_sb[:, k, r0:r0 + 256], psh[:],
                    AF.Relu, bias=b1_sb[:, k:k + 1],
                )
                for blk in blks:
                    for fo, sl in ((0, slice(0, 512)), (1, slice(512, 768))):
                        nc.tensor.matmul(
                            psf[(blk, fo)][:],
                            h_sb[:, k, blk * P:(blk + 1) * P],
                            w2_sb[:, k, sl],
                            start=(k == 0),
                            stop=(k == KO_F - 1),
                        )
            for blk in blks:
                for fo, sl in ((0, slice(0, 512)), (1, slice(512, 768))):
                    nc.vector.tensor_add(
                        u_sb[:, blk, sl], u_sb[:, blk, sl], psf[(blk, fo)][:]
                    )

        def strip_out(blk):
            ost = stage3.tile([P, D], F32, tag="ost")
            _layernorm(nc, lnsmall, ost[:], u_sb[:, blk, :], eps_sb, g2_sb, be2_sb)
            nc.gpsimd.dma_start(out[blk * P:(blk + 1) * P, :], ost[:])

        strip_ln1(0)
        strip_ln1(1)
        ffn_half(0)
        strip_ln1(2)
        strip_ln1(3)
        strip_out(0)
        strip_out(1)
        ffn_half(1)
        strip_out(2)
        strip_out(3)

        tpstack.close()
        frA.close()
        fr3.close()

    nc.compile()
    return nc


def _layernorm(nc, pool, zout, z, eps_sb, g_sb, b_sb):
    """zout = LayerNorm(z) * g + b over the free dim (768) of z [128, 768].

    rstd = exp(-0.5 * ln(var + eps)) keeps the Activation engine in the
    single function table that also holds Exp/Identity/Copy/Relu.
    The g-mul / b-add run on GpSimd (SBUF-only) to offload DVE.
    """
    sub = 256
    nsub = D // sub
    stats = pool.tile([P, nsub, nc.vector.BN_STATS_DIM], F32, tag="ln_stats")
    mv = pool.tile([P, nc.vector.BN_AGGR_DIM], F32, tag="ln_mv")
    zr = z.rearrange("p (n s) -> p n s", s=sub)
    for sg in range(nsub):
        nc.vector.bn_stats(stats[:, sg, :], zr[:, sg, :])
    nc.vector.bn_aggr(mv[:], stats[:])
    std = pool.tile([P, 1], F32, tag="ln_std")
    nc.scalar.activation(std[:], mv[:, 1:2], AF.Sqrt, bias=eps_sb[:])
    rstd = pool.tile([P, 1], F32, tag="ln_rstd")
    nc.vector.reciprocal(rstd[:], std[:])
    nc.vector.tensor_scalar(
        zout, z,
        scalar1=mv[:, 0:1],
        scalar2=rstd[:],
        op0=ALU.subtract,
        op1=ALU.mult,
    )
    nc.vector.tensor_mul(zout, zout, g_sb[:])
    nc.vector.tensor_add(zout, zout, b_sb[:])


def _host_inputs(x, wq, bq, wk, bk, wv, bv, wo, bo, w1, b1, w2, b2,
                 g1, be1, g2, be2):
    """Build the per-core input maps (bf16 weights, constant folds)."""
    f = np.float32
    bf = ml_dtypes.bfloat16
    ident = np.eye(P, dtype=f)
    # triangular diagonal-block mask: keep key p <= query q (within block)
    trimask = (np.arange(P)[:, None] <= np.arange(P)[None, :]).astype(bf)

    xT = [np.ascontiguousarray(x[b].T).astype(bf) for b in range(B)]

    scale = f(1.0 / np.sqrt(DK))
    # The on-chip u tile stores x1 + b2 (b2 rides on LN1's output bias so
    # LN2's residual input is ready); compensate in FFN1's bias.
    b1f = (b1 - b2 @ w1).astype(f)
    bebb = np.broadcast_to(be1 + b2, (P, D)).astype(f)
    resid_bias = (bo + bv @ wo).astype(f)  # bv folded through out-proj

    shared = {
        "w1": np.ascontiguousarray(w1).astype(bf),
        "b1": b1f,
        "w2": np.ascontiguousarray(w2).astype(bf),
        "g1b": np.broadcast_to(g1, (P, D)).astype(f),
        "beb2": bebb,
        "g2b": np.broadcast_to(g2, (P, D)).astype(f),
        "be2b": np.broadcast_to(be2, (P, D)).astype(f),
        "ident": ident,
        "ones3": (32 * (np.arange(HD)[None, :] // DK) == np.arange(DK + 1)[:, None]).astype(bf),
        "trimask": trimask,
    }

    in_maps = []
    for c in range(NCORES):
        b, r = divmod(c, RANKS)
        hs = slice(r * HD, (r + 1) * HD)
        wvp = wv[:, hs].astype(bf)
        # [q0 q1 | q2 pad | k0 k1 | k2 pad], q pre-scaled by 1/sqrt(dk)
        wqkp = np.zeros((D, 4 * P), f)
        wqkp[:, 0:P] = wq[:, hs][:, 0:P] * scale
        wqkp[:, P:P + DK] = wq[:, hs][:, P:HD] * scale
        wqkp[:, 2 * P:3 * P] = wk[:, hs][:, 0:P]
        wqkp[:, 3 * P:3 * P + DK] = wk[:, hs][:, P:HD]
        bqkp = np.zeros(4 * P, f)
        bqkp[0:P] = bq[hs][0:P] * scale
        bqkp[P:P + DK] = bq[hs][P:HD] * scale
        bqkp[2 * P:3 * P] = bk[hs][0:P]
        bqkp[3 * P:3 * P + DK] = bk[hs][P:HD]
        # owned rows: strip r of each 512-chunk, + (bo + bv@wo)
        xown = np.concatenate(
            [x[b, cc * 512 + r * P: cc * 512 + (r + 1) * P] for cc in range(QC)],
            axis=0,
        ) + resid_bias
        m = {
            "xbT": xT[b],
            "xown": np.ascontiguousarray(xown).astype(f),
            "wqk": wqkp.astype(bf),
            "bqk": bqkp,
            "wvp": wvp,
            "wo_s": np.ascontiguousarray(wo[hs, :]).astype(bf),
        }
        m.update(shared)
        in_maps.append({k: np.ascontiguousarray(v) for k, v in m.items()})
    return in_maps


def _get_nc():
    if "nc" not in _CACHE:
        _CACHE["nc"] = _build()
    return _CACHE["nc"]


def run(inputs, **kw):
    """Run on hardware; returns (output, BassKernelResults)."""
    nc = _get_nc()
    in_maps = _host_inputs(**inputs)
    res = run_bass_kernel_spmd(nc, in_maps, core_ids=list(range(NCORES)), **kw)
    out = np.empty((B, S, D), np.float32)
    for core in range(NCORES):
        b, r = divmod(core, RANKS)
        o = res.results[core]["out"]
        for c in range(QC):
            out[b, c * 512 + r * P: c * 512 + (r + 1) * P, :] = o[c * P:(c + 1) * P]
    return out, res


def kernel(**inputs):
    return run(inputs)[0]
